# revision 1
# baseline (speedup 1.0000x reference)
"""Trainium2 Bass kernel v2 for nn_NbrAttn2 (neighbor cross-attention block).

Sharding: 8 cores = 4 batches x 2 kv-halves. Each core computes attention
for its batch over half the neighbors (kv = 8*512 = 4096) for all 8 heads,
producing per-head unnormalized context + softmax denominators z. Only z
([8,512] = 16KB) is AllReduce-added within each pair; each core then
normalizes its own context partial with the global z, applies the output
projection, and writes a partial [T, D] output (even cores add residual
xq + bo). The host sums the pair partials.

Key optimizations vs v1: host-side bf16 mask (no int32 convert, half the
DMA), bf16 prep matmuls (4x PE throughput vs fp32), host-folded x+pe and
pe@Wk/pe@Wv terms, 2-neighbor-batched prep stages with relu+bias fused on
the scalar engine, head-major attention with PSUM-resident per-head ctx
accumulation, input DMAs spread across three descriptor queues, and a 34x
smaller collective (z only, [8x512] f32).
"""

import math

import numpy as np

B, T, N, D, H = 4, 512, 16, 256, 8
DK = D // H  # 32
HD = DK + 1  # ctx rows per head incl z
CTS, CN, CE = 6, 4, 3
TSE, AUXE = 192, 64
NCORES = 8
NBH = N // 2       # neighbors per core
KV = NBH * T       # 4096 kv positions per core
KC = KV // 128     # 32 kv chunks of 128

_CACHE = {}


def _pe_table() -> np.ndarray:
    # matches reference.pe_table numerics (fp32)
    pos = np.arange(T, dtype=np.float32)[:, None]
    div = np.exp(
        np.arange(0, D, 2, dtype=np.float32)
        * (np.float32(-np.log(np.float32(10000.0))) / np.float32(D))
    ).astype(np.float32)
    pe = np.zeros((T, D), dtype=np.float32)
    pe[:, 0::2] = np.sin(pos * div)
    pe[:, 1::2] = np.cos(pos * div)
    return pe


def _hmap(h):
    """head -> (tile index, partition offset) over 96/96/64 partition tiles."""
    return (h // 3, (h % 3) * DK) if h < 6 else (2, (h - 6) * DK)


def build_nc(loop: int = 0, no_collective: bool = False, phases: str = "mlpa"):
    import concourse.bass as bass  # noqa: F401
    import concourse.mybir as mybir
    import concourse.tile as tile
    from concourse import bacc
    from concourse.masks import make_identity

    f32 = mybir.dt.float32
    bf16 = mybir.dt.bfloat16
    AF = mybir.ActivationFunctionType
    OP = mybir.AluOpType

    nc = bacc.Bacc()

    dp = nc.declare_dram_parameter
    maskt_h = dp("maskt", [KV, T], bf16, isOutput=False)
    xq_h = dp("xq", [T, D], f32, isOutput=False)          # 0.5*(x+pe), host-folded
    # aux rows: 0-5 md, 32-35 na, 64-66 ea (32-aligned for matmul base rules)
    aux_h = dp("aux", [64 + CE, NBH * T], bf16, isOutput=False)
    # w1pack rows: 0-5 W_ts1, 32-35 W_a1, 64-66 W_e1
    w1pack_h = dp("w1pack", [64 + CE, D], bf16, isOutput=False)
    wts2_h = dp("wts2", [TSE, TSE], bf16, isOutput=False)
    wa2_h = dp("wa2", [AUXE, AUXE], bf16, isOutput=False)
    we2_h = dp("we2", [D, D], bf16, isOutput=False)
    wq_h = dp("wq", [D, D], bf16, isOutput=False)         # * 1/sqrt(DK)
    wk_h = dp("wk", [D, D], bf16, isOutput=False)
    wv_h = dp("wv", [D, D], bf16, isOutput=False)
    wo_h = dp("wo", [D + 1, D], bf16, isOutput=False)     # bias row (even core)
    biases_h = dp("biases", [6, D, 1], f32, isOutput=False)
    pewk_h = dp("pewk", [D, 2 * T], bf16, isOutput=False)  # (pe@Wk + bk).T, 2x tiled
    pewv_h = dp("pewv", [T, D], bf16, isOutput=False)      # pe@Wv + bv
    lng_h = dp("lng", [D, 1], f32, isOutput=False)
    lnb_h = dp("lnb", [D, 1], f32, isOutput=False)
    ind4_h = dp("ind4", [4, 128], f32, isOutput=False)    # block indicator
    out_h = dp("out", [T, D], f32, isOutput=True)

    RG = [[0, 1], [2, 3], [4, 5], [6, 7]]
    NP = NBH // 2  # neighbor pairs

    do_m = "m" in phases
    do_l = "l" in phases
    do_p = "p" in phases
    do_a = "a" in phases
    do_epi = phases == "mlpa" or "z" in phases

    with tile.TileContext(nc, num_cores=NCORES) as tc:
        with (
            tc.tile_pool(name="const", bufs=1) as const,
            tc.tile_pool(name="big", bufs=1) as big,
            tc.tile_pool(name="prep", bufs=2) as prep,
            tc.tile_pool(name="ppool", bufs=3) as ppool,
            tc.tile_pool(name="dram", bufs=1, space="DRAM") as dram,
        ):
            # ---------------- constants ----------------
            ident = const.tile([128, 128], f32, name="ident")
            make_identity(nc, ident[:])
            ones_row = const.tile([1, T], bf16, name="ones_row")
            nc.vector.memset(ones_row[:], 1.0)
            ones_f32 = const.tile([1, T], f32, name="ones_f32")
            nc.vector.memset(ones_f32[:], 1.0)
            # xq arrives host-halved (0.5*(x+pe)); LN is scale-invariant but
            # eps must scale by 0.25 to match reference numerics exactly.
            eps_col = const.tile([128, 1], f32, name="eps_col")
            nc.vector.memset(eps_col[:], 0.25e-6)
            # 4-head block indicator for z-broadcast: ind4[h, 32h:32h+32]=1
            ind4 = const.tile([4, 128], f32, name="ind4")
            nc.sync.dma_start(out=ind4[:], in_=ind4_h[:])

            _dma_rr = [nc.sync, nc.scalar, nc.gpsimd]

            def load_const(name, src, p, f, dt=bf16, q=None):
                t = const.tile([p, f], dt, name=name)
                eng = _dma_rr[q % 3] if q is not None else nc.sync
                eng.dma_start(out=t[:], in_=src)
                return t

            # prep-critical inputs first so the SP DMA queue unblocks compute
            aux_all = const.tile([64 + CE, NBH * T], bf16, name="aux_all")
            nc.sync.dma_start(out=aux_all[:], in_=aux_h[:])
            md_all = aux_all[0:CTS]
            na_all = aux_all[32 : 32 + CN]
            ea_all = aux_all[64 : 64 + CE]
            w1pack = load_const("w1pack", w1pack_h[:], 64 + CE, D)
            w_ts1 = w1pack[0:CTS, 0:TSE]
            w_a1 = w1pack[32 : 32 + CN, 0:AUXE]
            w_e1 = w1pack[64 : 64 + CE, 0:D]
            w_ts2 = [load_const(f"w_ts2_{c}", wts2_h[c * 128 : min(TSE, (c + 1) * 128)],
                                min(128, TSE - c * 128), TSE) for c in range(2)]
            w_a2 = load_const("w_a2", wa2_h[:], AUXE, AUXE)
            w_e2 = [load_const(f"w_e2_{c}", we2_h[c * 128 : (c + 1) * 128], 128, D)
                    for c in range(2)]
            w_q = [load_const(f"w_q_{c}", wq_h[c * 128 : (c + 1) * 128], 128, D, q=2)
                   for c in range(2)]
            w_k = [load_const(f"w_k_{c}", wk_h[c * 128 : (c + 1) * 128], 128, D, q=2)
                   for c in range(2)]
            w_v = [load_const(f"w_v_{c}", wv_h[c * 128 : (c + 1) * 128], 128, D, q=2)
                   for c in range(2)]
            w_o = [load_const(f"w_o_{c}", wo_h[c * 128 : (c + 1) * 128], 128, D, q=2)
                   for c in range(2)]
            w_o_b = load_const("w_o_b", wo_h[256:257], 1, D, q=1)
            pewk = [load_const(f"pewk{c}", pewk_h[c * 128 : (c + 1) * 128], 128, 2 * T, q=2)
                    for c in range(2)]
            pewv = [load_const(f"pewv{j}", pewv_h[j * 128 : (j + 1) * 128], 128, D, q=2)
                    for j in range(4)]
            # bias columns: 0=b_ts1, 1=b_ts2, 2=b_a (a1 0:64, a2 64:128),
            # 3=b_e1, 4=b_e2, 5=bq/sqrt(DK)
            _bc = [(load_const(f"bcol{i}a", biases_h[i, 0:128], 128, 1, f32, q=1),
                    load_const(f"bcol{i}b", biases_h[i, 128:256], 128, 1, f32, q=1))
                   for i in range(6)]

            class _BCol:
                def __init__(self, pair):
                    self.pair = pair

                def __getitem__(self, s):
                    lo, hi = s.start or 0, s.stop
                    c, r = divmod(lo, 128)
                    assert hi - lo <= 128 - r
                    return self.pair[c][r : r + (hi - lo)]

            bcol = [_BCol(p) for p in _bc]
            lng = [load_const(f"lng{c}", lng_h[c * 128 : (c + 1) * 128], 128, 1, f32, q=1)
                   for c in range(2)]
            lnb = [load_const(f"lnb{c}", lnb_h[c * 128 : (c + 1) * 128], 128, 1, f32, q=1)
                   for c in range(2)]
            # ---------------- persistent big tensors ----------------
            mask_bf = big.tile([128, KC * T], bf16, name="mask_bf")
            kT_bf = [big.tile([96, KV], bf16, name="kT_a"),
                     big.tile([96, KV], bf16, name="kT_b"),
                     big.tile([64, KV], bf16, name="kT_c")]
            qT_bf = [big.tile([96, T], bf16, name="qT_a"),
                     big.tile([96, T], bf16, name="qT_b"),
                     big.tile([64, T], bf16, name="qT_c")]
            v_aug = big.tile([128, KC * H * HD], bf16, name="v_aug")
            xq_sb = [big.tile([128, D], f32, name=f"xq{t}") for t in range(4)]
            xnT_bf = [big.tile([128, T], bf16, name=f"xnT{c}") for c in range(2)]
            ctxT_sb = [big.tile([128, T], f32, name=f"ctxT{c}") for c in range(2)]
            # z for head h lives at partition 32*(h%4), col-half h//4
            z_tile = big.tile([128, 2 * T], f32, name="z_tile")

            def zoff(h):
                return ((h % 4) * 2 + h // 4) * T

            va4 = v_aug.rearrange("p (c h e) -> p c h e", c=KC, h=H)
            nc.vector.memset(va4[:, :, :, DK : DK + 1], 1.0)
            va3 = v_aug.rearrange("p (c e) -> p c e", c=KC)  # [128, KC, 264]

            if do_a and not do_p:
                for t_ in kT_bf + qT_bf:
                    nc.vector.memset(t_[:], 0.5)
                nc.vector.memset(v_aug[:], 0.5)
                nc.vector.memset(va4[:, :, :, DK : DK + 1], 1.0)
            if do_a and not do_m:
                nc.vector.memset(mask_bf[:], 1.0)
            if do_epi and not do_a:
                nc.vector.memset(z_tile[:], 1.0)
                for t_ in ctxT_sb:
                    nc.vector.memset(t_[:], 0.5)
            if do_epi and not do_l:
                for t_ in xq_sb:
                    nc.vector.memset(t_[:], 0.0)

            for _rep in range(max(1, loop)):
                with tc.tile_pool(name="pp", bufs=1, space="PSUM") as pp:
                    # ====== phase A: xq + mask DMAs, LN, q ======
                    if do_l:
                        for t in range(4):
                            nc.sync.dma_start(
                                out=xq_sb[t][:], in_=xq_h[t * 128 : (t + 1) * 128]
                            )
                    if do_m:
                        mT = maskt_h.rearrange("(c p) t -> p c t", p=128)
                        m3 = mask_bf.rearrange("p (c t) -> p c t", c=KC)
                        for mc in range(4):
                            nc.gpsimd.dma_start(
                                out=m3[:, mc * 8 : (mc + 1) * 8, :],
                                in_=mT[:, mc * 8 : (mc + 1) * 8, :],
                            )

                    for t in range(4 if do_l else 0):
                        xt = xq_sb[t]
                        mu = prep.tile([128, 1], f32, name="ln_mu", tag="lncol", bufs=8)
                        nc.vector.tensor_reduce(
                            mu[:], xt[:], mybir.AxisListType.X, OP.add
                        )
                        nc.vector.tensor_scalar_mul(mu[:], mu[:], 1.0 / D)
                        xc = prep.tile([128, D], f32, name="ln_xc", tag="lnw", bufs=4)
                        nc.vector.tensor_scalar(xc[:], xt[:], mu[:], None, OP.subtract)
                        sq = prep.tile([128, D], f32, name="ln_sq", tag="lnw", bufs=4)
                        var = prep.tile([128, 1], f32, name="ln_var", tag="lncol", bufs=8)
                        nc.scalar.activation(sq[:], xc[:], AF.Square, accum_out=var[:])
                        std = prep.tile([128, 1], f32, name="ln_std", tag="lncol", bufs=8)
                        nc.scalar.activation(
                            std[:], var[:], AF.Sqrt, bias=eps_col[:], scale=1.0 / D
                        )
                        rstd = prep.tile([128, 1], f32, name="ln_rstd", tag="lncol", bufs=8)
                        nc.vector.reciprocal(rstd[:], std[:])
                        xn0 = prep.tile([128, D], f32, name="ln_xn0", tag="lnw", bufs=4)
                        nc.vector.tensor_scalar_mul(xn0[:], xc[:], rstd[:])
                        for c in range(2):
                            tp = pp.tile([128, 512], f32, name="tp", tag="v", bufs=2)
                            nc.tensor.transpose(
                                tp[:, 0:128], xn0[:, c * 128 : (c + 1) * 128], ident[:]
                            )
                            nc.vector.tensor_scalar(
                                xnT_bf[c][:, t * 128 : (t + 1) * 128],
                                tp[:, 0:128], lng[c][:], lnb[c][:], OP.mult, OP.add,
                            )
                    for g in range(2 if do_l else 0):
                        qp = pp.tile([128, 1024], f32, name="qp", tag="st", bufs=3)
                        gs = slice(g * 128, (g + 1) * 128)
                        nc.tensor.matmul(qp[:, 0:T], w_q[0][:, gs], xnT_bf[0][:],
                                         start=True, stop=False)
                        nc.tensor.matmul(qp[:, 0:T], w_q[1][:, gs], xnT_bf[1][:],
                                         start=False, stop=True)
                        bq_ = bcol[5]
                        if g == 0:
                            nc.vector.tensor_scalar_add(qT_bf[0][:], qp[0:96, 0:T], bq_[0:96])
                            nc.vector.tensor_scalar_add(qT_bf[1][0:32, :], qp[96:128, 0:T], bq_[96:128])
                        else:
                            nc.vector.tensor_scalar_add(qT_bf[1][32:64, :], qp[0:32, 0:T], bq_[128:160])
                            nc.vector.tensor_scalar_add(qT_bf[1][64:96, :], qp[32:64, 0:T], bq_[160:192])
                            nc.vector.tensor_scalar_add(qT_bf[2][:], qp[64:128, 0:T], bq_[192:256])

                    # ====== phases B+C interleaved: per-pair prep, then its
                    # attention chunk-groups (prep of pair p+1 overlaps
                    # attention of pair p on disjoint engines) ======
                    def mm2(ps, terms, rows=slice(0, 128)):
                        """Accumulate matmul terms into both 512-halves of a
                        [128,1024] psum tile (one bank per matmul)."""
                        for j in range(2):
                            for idx, (lh, mv) in enumerate(terms):
                                nc.tensor.matmul(
                                    ps[rows, j * T : (j + 1) * T],
                                    lh, mv[:, j * T : (j + 1) * T],
                                    start=(idx == 0), stop=(idx == len(terms) - 1),
                                )

                    for np_ in range(NP if do_p else 0):
                        n0 = 2 * np_
                        nts2 = slice(n0 * T, (n0 + 2) * T)  # 2-neighbor slice
                        if True:
                            md2 = md_all[:, nts2]
                            na2 = na_all[:, nts2]
                            ea2 = ea_all[:, nts2]

                            # ts branch: ts1 = relu(W_ts1^T md + b)
                            ts1 = [prep.tile([128, 1024], bf16, name=f"ts1_{c}",
                                             tag=f"ts1{c}", bufs=2) for c in range(2)]
                            ps = pp.tile([128, 1024], f32, name="ps_ts1a", tag="st", bufs=3)
                            mm2(ps, [(w_ts1[:, 0:128], md2)])
                            nc.scalar.activation(ts1[0][:], ps[:], AF.Relu, bias=bcol[0][0:128])
                            ps = pp.tile([128, 1024], f32, name="ps_ts1b", tag="st", bufs=3)
                            mm2(ps, [(w_ts1[:, 128:TSE], md2)], rows=slice(0, TSE - 128))
                            nc.scalar.activation(
                                ts1[1][0 : TSE - 128, :], ps[0 : TSE - 128, :],
                                AF.Relu, bias=bcol[0][128:TSE])
                            # aux branch: a1 = relu(W_a1^T na + b)
                            a1 = prep.tile([128, 1024], bf16, name="a1sb", tag="a1", bufs=2)
                            ps = pp.tile([128, 1024], f32, name="ps_a1", tag="st", bufs=3)
                            mm2(ps, [(w_a1[:], na2)], rows=slice(0, AUXE))
                            nc.scalar.activation(
                                a1[0:AUXE, :], ps[0:AUXE, :], AF.Relu, bias=bcol[2][0:AUXE])

                            # nbr = [ts2 (192) ; a2 (64)] as 2 chunks of 128
                            nbr = [prep.tile([128, 1024], bf16, name=f"nbr_{c}",
                                             tag=f"nbr{c}", bufs=2) for c in range(2)]
                            ps = pp.tile([128, 1024], f32, name="ps_nbr0", tag="st", bufs=3)
                            mm2(ps, [(w_ts2[0][:, 0:128], ts1[0]),
                                     (w_ts2[1][0:64, 0:128], ts1[1][0:64, :])])
                            nc.vector.tensor_scalar_add(nbr[0][:], ps[:], bcol[1][0:128])
                            ps = pp.tile([128, 1024], f32, name="ps_nbr1", tag="st", bufs=3)
                            mm2(ps, [(w_ts2[0][:, 128:TSE], ts1[0]),
                                     (w_ts2[1][0:64, 128:TSE], ts1[1][0:64, :])],
                                rows=slice(0, 64))
                            nc.vector.tensor_scalar_add(
                                nbr[1][0:64, :], ps[0:64, :], bcol[1][128:TSE])
                            ps = pp.tile([128, 1024], f32, name="ps_a2", tag="st", bufs=3)
                            mm2(ps, [(w_a2[:], a1[0:AUXE, :])], rows=slice(0, AUXE))
                            nc.vector.tensor_scalar_add(
                                nbr[1][64:128, :], ps[0:AUXE, :], bcol[2][64:128])

                            # edge branch
                            e1 = [prep.tile([128, 1024], bf16, name=f"e1_{c}",
                                            tag=f"e1{c}", bufs=2) for c in range(2)]
                            for c in range(2):
                                ps = pp.tile([128, 1024], f32, name="ps_e1", tag="st", bufs=3)
                                mm2(ps, [(w_e1[:, c * 128 : (c + 1) * 128], ea2)])
                                nc.scalar.activation(
                                    e1[c][:], ps[:], AF.Relu,
                                    bias=bcol[3][c * 128 : (c + 1) * 128])
                            e2 = [prep.tile([128, 1024], bf16, name=f"e2_{c}",
                                            tag=f"e2{c}", bufs=2) for c in range(2)]
                            for c in range(2):
                                cs = slice(c * 128, (c + 1) * 128)
                                ps = pp.tile([128, 1024], f32, name="ps_e2", tag="st", bufs=3)
                                mm2(ps, [(w_e2[0][:, cs], e1[0]), (w_e2[1][:, cs], e1[1])])
                                nc.vector.tensor_scalar_add(e2[c][:], ps[:], bcol[4][cs])

                            # keysT = nbr * e2 (pe+bias via pewk const)
                            keys = [prep.tile([128, 1024], bf16, name=f"keys_{c}",
                                              tag=f"keys{c}", bufs=1) for c in range(2)]
                            nc.gpsimd.tensor_tensor(keys[0][:], nbr[0][:], e2[0][:], OP.mult)
                            nc.gpsimd.tensor_tensor(keys[1][:], nbr[1][:], e2[1][:], OP.mult)

                            # kT = Wk^T keys + pewk^T  (bf16, 96/96/64 tiles)
                            nts = slice(n0 * T, (n0 + 2) * T)
                            for g in range(2):
                                gs = slice(g * 128, (g + 1) * 128)
                                ps = pp.tile([128, 1024], f32, name="ps_kt", tag="st", bufs=3)
                                mm2(ps, [(w_k[0][:, gs], keys[0]), (w_k[1][:, gs], keys[1])])
                                if g == 0:
                                    nc.vector.tensor_tensor(
                                        kT_bf[0][:, nts], ps[0:96, :], pewk[0][0:96, :], OP.add)
                                    nc.vector.tensor_tensor(
                                        kT_bf[1][0:32, nts], ps[96:128, :], pewk[0][96:128, :], OP.add)
                                else:
                                    nc.vector.tensor_tensor(
                                        kT_bf[1][32:64, nts], ps[0:32, :], pewk[1][0:32, :], OP.add)
                                    nc.vector.tensor_tensor(
                                        kT_bf[1][64:96, nts], ps[32:64, :], pewk[1][32:64, :], OP.add)
                                    nc.vector.tensor_tensor(
                                        kT_bf[2][:, nts], ps[64:128, :], pewk[1][64:128, :], OP.add)

                            # v rows: v = nbr^T Wv + pewv -> va (aug layout); two
                            # [128,256] outputs per [128,512] psum tile (1 bank)
                            for i in range(2):
                                for j2 in range(2):
                                    ps = pp.tile([128, 512], f32, name="ps_v", tag="v", bufs=2)
                                    for jj in range(2):
                                        j = 2 * j2 + jj
                                        ts_ = slice(i * 512 + j * 128, i * 512 + (j + 1) * 128)
                                        nc.tensor.matmul(
                                            ps[:, jj * D : (jj + 1) * D], nbr[0][:, ts_],
                                            w_v[0][:], start=True, stop=False)
                                        nc.tensor.matmul(
                                            ps[:, jj * D : (jj + 1) * D], nbr[1][:, ts_],
                                            w_v[1][:], start=False, stop=True)
                                    for jj in range(2):
                                        j = 2 * j2 + jj
                                        kc = (n0 + i) * 4 + j
                                        nc.vector.tensor_tensor(
                                            va4[:, kc, :, 0:DK],
                                            ps[:, jj * D : (jj + 1) * D].rearrange(
                                                "p (h e) -> p h e", h=H),
                                            pewv[j][:].rearrange("p (h e) -> p h e", h=H),
                                            OP.add,
                                        )

                # prefetch the exp table set while prep drains
                if do_a:
                    dummy_exp = prep.tile([128, 1], f32, name="dummy_exp", tag="lncol", bufs=8)
                    nc.scalar.activation(dummy_exp[:], eps_col[:], AF.Exp)

                # ====== phase C: head-major attention ======
                if do_a:
                    with (
                        tc.tile_pool(name="psc", bufs=2, space="PSUM") as psc,
                        tc.tile_pool(name="pcx", bufs=2, space="PSUM") as pcx,
                    ):
                        for h in range(H):
                            g, hoff = _hmap(h)
                            krows = slice(hoff, hoff + DK)
                            cxt = pcx.tile([128, T], f32, name="cxt", tag="cx", bufs=2)
                            GRP = [(s, min(3, KC - s)) for s in range(0, KC, 3)]
                            for c0, gn in GRP:
                                sp = psc.tile([128, 3 * T], f32, name="sp", tag="sp", bufs=2)
                                for j in range(gn):
                                    kc = c0 + j
                                    nc.tensor.matmul(
                                        sp[:, j * T : (j + 1) * T],
                                        kT_bf[g][krows, kc * 128 : (kc + 1) * 128],
                                        qT_bf[g][krows, :],
                                        start=True, stop=True,
                                    )
                                p0 = ppool.tile([128, 3 * T], bf16, name="p0",
                                                tag="p0", bufs=3)
                                nc.scalar.activation(p0[:, 0 : gn * T], sp[:, 0 : gn * T], AF.Exp)
                                pm = ppool.tile([128, 3 * T], bf16, name="pm",
                                                tag="pm", bufs=4)
                                nc.vector.tensor_tensor(
                                    pm[:, 0 : gn * T], p0[:, 0 : gn * T],
                                    mask_bf[:, c0 * T : (c0 + gn) * T],
                                    OP.mult,
                                )
                                for j in range(gn):
                                    kc = c0 + j
                                    nc.tensor.matmul(
                                        cxt[0:HD, :],
                                        va3[:, kc, h * HD : (h + 1) * HD],
                                        pm[:, j * T : (j + 1) * T],
                                        start=(kc == 0),
                                        stop=(kc == KC - 1),
                                    )
                            # extract ctx + z for this head
                            c4, r4 = divmod(h, 4)
                            nc.vector.tensor_copy(
                                ctxT_sb[c4][r4 * DK : (r4 + 1) * DK, :], cxt[0:DK, :]
                            )
                            nc.vector.tensor_copy(
                                z_tile[r4 * DK : r4 * DK + 1, c4 * T : (c4 + 1) * T],
                                cxt[DK : DK + 1, :],
                            )

                # ====== phase D: z AllReduce + epilogue ======
                if do_epi:
                    with tc.tile_pool(name="px", bufs=2, space="PSUM") as px:
                        cc_in = dram.tile([H * T], f32, name="cc_in")
                        cc_out = dram.tile([H * T], f32, name="cc_out")
                        zv = z_tile.rearrange("(a b) (c t) -> a b c t", b=32, c=2)
                        nc.sync.dma_start(
                            out=cc_in.rearrange("(c a t) -> a c t", a=4, c=2),
                            in_=zv[:, 0, :, :],
                        )
                        if no_collective:
                            nc.sync.dma_start(out=cc_out[:], in_=cc_in[:])
                        else:
                            nc.gpsimd.collective_compute(
                                "AllReduce", OP.add, replica_groups=RG,
                                ins=[cc_in[:]], outs=[cc_out[:]],
                            )
                        zg = big.tile([1, H * T], f32, name="zg")
                        nc.sync.dma_start(
                            out=zg[:], in_=cc_out.rearrange("(o t) -> o t", o=1)
                        )
                        rzf = big.tile([1, H * T], bf16, name="rzf")
                        ctxn = [big.tile([128, T], bf16, name=f"ctxn{c}") for c in range(2)]
                        with nc.allow_low_precision(reason="1/z in bf16; rel tol 2e-2"):
                            for c4 in range(2):
                                hs = slice(c4 * 4 * T, (c4 + 1) * 4 * T)
                                nc.vector.reciprocal(rzf[0:1, hs], zg[0:1, hs])
                            for h in range(H):
                                c4, r4 = divmod(h, 4)
                                bc = px.tile([128, T], f32, name="bc", tag="px")
                                nc.tensor.matmul(
                                    bc[0:DK, :], ones_row[0:1, 0:DK],
                                    rzf[0:1, h * T : (h + 1) * T],
                                    start=True, stop=True,
                                )
                                nc.vector.tensor_tensor(
                                    ctxn[c4][r4 * DK : (r4 + 1) * DK, :],
                                    ctxT_sb[c4][r4 * DK : (r4 + 1) * DK, :],
                                    bc[0:DK, :], OP.mult)
                        for t in range(4):
                            ts_ = slice(t * 128, (t + 1) * 128)
                            op_ = px.tile([128, D], f32, name="op", tag="po")
                            nc.tensor.matmul(op_[:], ctxn[0][:, ts_], w_o[0][:], start=True, stop=False)
                            nc.tensor.matmul(op_[:], ctxn[1][:, ts_], w_o[1][:], start=False, stop=False)
                            nc.tensor.matmul(op_[:], ones_row[0:1, ts_], w_o_b[:], start=False, stop=True)
                            ot = prep.tile([128, D], f32, name="out_sb", tag="lnw", bufs=4)
                            nc.vector.tensor_add(ot[:], op_[:], xq_sb[t][:])
                            nc.sync.dma_start(out=out_h[ts_, :], in_=ot[:])

    nc.finalize()
    return nc


def _host_inputs(inputs):
    """Build the 8 per-core input maps from full inputs."""
    import ml_dtypes

    bf16 = ml_dtypes.bfloat16
    pe = _pe_table()
    sc = np.float32(1.0 / math.sqrt(DK))

    w = {k: np.asarray(v, dtype=np.float32) if np.asarray(v).dtype != np.int32
         else np.asarray(v) for k, v in inputs.items()}

    def pad_col(v):
        out = np.zeros((D, 1), np.float32)
        out[: v.shape[0], 0] = v
        return out

    biases = np.stack([
        pad_col(w["b_ts1"]),
        pad_col(w["b_ts2"]),
        pad_col(np.concatenate([w["b_a1"], w["b_a2"]])),
        pad_col(w["b_e1"]),
        pad_col(w["b_e2"]),
        pad_col(w["bq"] * sc),
    ])

    w1pack = np.zeros((64 + CE, D), np.float32)
    w1pack[0:CTS, 0:TSE] = w["W_ts1"]
    w1pack[32 : 32 + CN, 0:AUXE] = w["W_a1"]
    w1pack[64 : 64 + CE, 0:D] = w["W_e1"]

    shared = {
        "w1pack": w1pack.astype(bf16),
        "wts2": w["W_ts2"].astype(bf16),
        "wa2": w["W_a2"].astype(bf16),
        "we2": w["W_e2"].astype(bf16),
        "wq": (w["Wq"] * sc).astype(bf16),
        "wk": w["Wk"].astype(bf16),
        "wv": w["Wv"].astype(bf16),
        "biases": biases,
        "pewk": np.ascontiguousarray(
            np.tile((pe @ w["Wk"] + w["bk"]).T, (1, 2))
        ).astype(bf16),
        "pewv": (pe @ w["Wv"] + w["bv"]).astype(bf16),
        "lng": w["ln_g"].reshape(D, 1).astype(np.float32),
        "lnb": w["ln_b"].reshape(D, 1).astype(np.float32),
        "ind4": np.kron(np.eye(4, dtype=np.float32), np.ones((1, DK), np.float32)),
    }
    wo_even = np.concatenate([w["Wo"], w["bo"][None, :]], axis=0).astype(bf16)
    wo_odd = np.concatenate([w["Wo"], np.zeros((1, D), np.float32)], axis=0).astype(bf16)

    in_maps = []
    for c in range(NCORES):
        b, half = divmod(c, 2)
        n0 = half * NBH
        m = dict(shared)
        # half-scaled: LN is scale-invariant (with eps/4 on device) and both
        # pair cores add it as residual, so the host pair-sum restores 1.0x.
        m["xq"] = (0.5 * (w["x"][b] + pe)).astype(np.float32)
        aux = np.zeros((64 + CE, NBH * T), np.float32)
        aux[0:CTS] = w["masked_data"][b, n0 : n0 + NBH].transpose(1, 0, 2).reshape(CTS, -1)
        aux[32 : 32 + CN] = w["node_aux"][b, n0 : n0 + NBH].transpose(1, 0, 2).reshape(CN, -1)
        aux[64 : 64 + CE] = w["edge_aux"][b, n0 : n0 + NBH].transpose(1, 0, 2).reshape(CE, -1)
        m["aux"] = aux.astype(bf16)
        m["maskt"] = np.ascontiguousarray(
            w["attention_mask"][b, :, half * KV : (half + 1) * KV].T
        ).astype(bf16)
        m["wo"] = wo_even if half == 0 else wo_odd
        in_maps.append(m)
    return in_maps


def _get_nc():
    if "nc" not in _CACHE:
        _CACHE["nc"] = build_nc()
    return _CACHE["nc"]


def kernel(**inputs) -> np.ndarray:
    from concourse.bass_utils import run_bass_kernel_spmd

    nc = _get_nc()
    in_maps = _host_inputs(inputs)
    res = run_bass_kernel_spmd(nc, in_maps, list(range(NCORES)))
    out = np.stack(
        [res.results[2 * b]["out"] + res.results[2 * b + 1]["out"] for b in range(B)],
        axis=0,
    )
    return out.astype(np.float32)



# revision 9
# speedup vs baseline: 1.2966x; 1.2966x over previous
"""Trainium2 Bass kernel v3 for nn_NbrAttn2 (neighbor cross-attention block).

Sharding: 8 cores = 4 batches x 2 kv-halves (unchanged from v2). Each core
computes attention for its batch over half the neighbors (KV = 8*512 = 4096)
for all 8 heads; softmax denominators z ([8,512] f32) are AllReduce-added
within each pair; each core normalizes its context partial, applies the
output projection, and writes a partial [T, D] output that the host pair-sums.

v3 changes (engine rebalance, attention is ACT-exp-bound):
- scores matmuls (K=DK=32) packed 2x via tile_position row-groups; ctx
  matmuls (M=HD=33) packed 2x via col-groups -> PE attention time ~4x down.
- attention pipelined per kv-chunk in 2-head groups: scores [128,1024] psum
  (2 banks, double-buffered) -> one exp per group -> [128,2048] masked mult
  on DVE (2x bf16 mode, mask broadcast over heads) -> col-packed ctx into 4
  persistent psum banks (head pair per bank, z row included at M=33).
- single ACT table set (natural_log_exp_and_others): LN rstd computed as
  exp(-0.5*ln(var+eps)) instead of Sqrt+reciprocal -> no table thrash.
- kT/qT stored as [128, KV]/[128, T] per 128-dim group (head h at partition
  32*(h%4)), making prep epilogues single big DVE ops and giving the packed
  scores their stationary layout for free.
- v += pe@Wv folded in 4-dim-AP DVE ops (4 instead of 16 per pair); mask
  DMA'd in the on-chip [128, KC*T] layout (128 contiguous 8KB descriptors)
  on the sync queue, freeing the gpsimd engine.
- z rows DMA'd straight from PSUM to the collective buffer; 1/z computed as
  [8,512] reciprocal (partition-parallel), broadcast per head with a tiny
  K=8 indicator matmul.
"""

import math

import numpy as np

B, T, N, D, H = 4, 512, 16, 256, 8
DK = D // H  # 32
HD = DK + 1  # ctx rows per head incl z
CTS, CN, CE = 6, 4, 3
TSE, AUXE = 192, 64
NCORES = 8
NBH = N // 2       # neighbors per core
KV = NBH * T       # 4096 kv positions per core
KC = KV // 128     # 32 kv chunks of 128

_CACHE = {}


def _pe_table() -> np.ndarray:
    # matches reference.pe_table numerics (fp32)
    pos = np.arange(T, dtype=np.float32)[:, None]
    div = np.exp(
        np.arange(0, D, 2, dtype=np.float32)
        * (np.float32(-np.log(np.float32(10000.0))) / np.float32(D))
    ).astype(np.float32)
    pe = np.zeros((T, D), dtype=np.float32)
    pe[:, 0::2] = np.sin(pos * div)
    pe[:, 1::2] = np.cos(pos * div)
    return pe


def build_nc(loop: int = 0, no_collective: bool = False, phases: str = "mlpa"):
    import concourse.bass as bass  # noqa: F401
    import concourse.mybir as mybir
    import concourse.tile as tile
    from concourse import bacc
    from concourse.masks import make_identity

    f32 = mybir.dt.float32
    bf16 = mybir.dt.bfloat16
    AF = mybir.ActivationFunctionType
    OP = mybir.AluOpType

    nc = bacc.Bacc()

    dp = nc.declare_dram_parameter
    maskt_h = dp("maskt", [128, KC * T], bf16, isOutput=False)  # on-chip layout
    xq_h = dp("xq", [T, D], f32, isOutput=False)          # 0.5*(x+pe), host-folded
    # aux rows: 0-5 md, 32-35 na, 64-66 ea (32-aligned for matmul base rules)
    aux_h = dp("aux", [64 + CE, NBH * T], bf16, isOutput=False)
    # w1pack rows: 0-5 W_ts1, 32-35 W_a1, 64-66 W_e1
    w1pack_h = dp("w1pack", [64 + CE, D], bf16, isOutput=False)
    wts2_h = dp("wts2", [TSE, TSE], bf16, isOutput=False)
    wa2_h = dp("wa2", [AUXE, AUXE], bf16, isOutput=False)
    we2_h = dp("we2", [D, D], bf16, isOutput=False)
    wq_h = dp("wq", [D, D], bf16, isOutput=False)         # * 1/sqrt(DK)
    wk_h = dp("wk", [D, D], bf16, isOutput=False)
    wv_h = dp("wv", [D, D], bf16, isOutput=False)
    wo_h = dp("wo", [D + 1, D], bf16, isOutput=False)     # bias row (even core)
    biases_h = dp("biases", [6, D, 1], f32, isOutput=False)
    pewk_h = dp("pewk", [D, 2 * T], bf16, isOutput=False)  # (pe@Wk + bk).T, 2x tiled
    pewv4_h = dp("pewv4", [128, 4 * D], bf16, isOutput=False)  # (pe@Wv+bv) t-blocked
    lng_h = dp("lng", [D, 1], f32, isOutput=False)
    lnb_h = dp("lnb", [D, 1], f32, isOutput=False)
    ind8_h = dp("ind8", [8, H * DK], bf16, isOutput=False)  # head indicator
    out_h = dp("out", [T, D], f32, isOutput=True)

    RG = [[0, 1], [2, 3], [4, 5], [6, 7]]
    NP = NBH // 2  # neighbor pairs

    do_m = "m" in phases
    do_l = "l" in phases
    do_p = "p" in phases
    do_a = "a" in phases
    do_epi = phases == "mlpa" or "z" in phases

    with tile.TileContext(nc, num_cores=NCORES) as tc:
        with (
            tc.tile_pool(name="const", bufs=1) as const,
            tc.tile_pool(name="big", bufs=1) as big,
            tc.tile_pool(name="prep", bufs=2) as prep,
            tc.tile_pool(name="ppool", bufs=3) as ppool,
            tc.tile_pool(name="dram", bufs=1, space="DRAM") as dram,
        ):
            # ---------------- constants ----------------
            ident = const.tile([128, 128], f32, name="ident")
            make_identity(nc, ident[:])
            ones_row = const.tile([1, T], bf16, name="ones_row")
            nc.vector.memset(ones_row[:], 1.0)
            # xq arrives host-halved (0.5*(x+pe)); LN is scale-invariant but
            # eps must scale by 0.25 to match reference numerics exactly.
            eps_col = const.tile([128, 1], f32, name="eps_col")
            nc.vector.memset(eps_col[:], 0.25e-6)
            ind8 = const.tile([8, H * DK], bf16, name="ind8")
            nc.scalar.dma_start(out=ind8[:], in_=ind8_h[:])

            _dma_rr = [nc.sync, nc.scalar, nc.gpsimd]

            def load_const(name, src, p, f, dt=bf16, q=None):
                t = const.tile([p, f], dt, name=name)
                eng = _dma_rr[q % 3] if q is not None else nc.sync
                eng.dma_start(out=t[:], in_=src)
                return t

            # prep-critical inputs first so the SP DMA queue unblocks compute
            aux_all = const.tile([64 + CE, NBH * T], bf16, name="aux_all")
            nc.sync.dma_start(out=aux_all[:], in_=aux_h[:])
            md_all = aux_all[0:CTS]
            na_all = aux_all[32 : 32 + CN]
            ea_all = aux_all[64 : 64 + CE]
            w1pack = load_const("w1pack", w1pack_h[:], 64 + CE, D)
            w_ts1 = w1pack[0:CTS, 0:TSE]
            w_a1 = w1pack[32 : 32 + CN, 0:AUXE]
            w_e1 = w1pack[64 : 64 + CE, 0:D]
            w_ts2 = [load_const(f"w_ts2_{c}", wts2_h[c * 128 : min(TSE, (c + 1) * 128)],
                                min(128, TSE - c * 128), TSE) for c in range(2)]
            w_a2 = load_const("w_a2", wa2_h[:], AUXE, AUXE)
            w_e2 = [load_const(f"w_e2_{c}", we2_h[c * 128 : (c + 1) * 128], 128, D)
                    for c in range(2)]
            w_q = [load_const(f"w_q_{c}", wq_h[c * 128 : (c + 1) * 128], 128, D, q=2)
                   for c in range(2)]
            w_k = [load_const(f"w_k_{c}", wk_h[c * 128 : (c + 1) * 128], 128, D, q=2)
                   for c in range(2)]
            w_v = [load_const(f"w_v_{c}", wv_h[c * 128 : (c + 1) * 128], 128, D, q=2)
                   for c in range(2)]
            w_o = [load_const(f"w_o_{c}", wo_h[c * 128 : (c + 1) * 128], 128, D, q=2)
                   for c in range(2)]
            w_o_b = load_const("w_o_b", wo_h[256:257], 1, D, q=1)
            pewk = [load_const(f"pewk{c}", pewk_h[c * 128 : (c + 1) * 128], 128, 2 * T, q=2)
                    for c in range(2)]
            pewv4 = load_const("pewv4", pewv4_h[:], 128, 4 * D, q=2)
            # bias columns: 0=b_ts1, 1=b_ts2, 2=b_a (a1 0:64, a2 64:128),
            # 3=b_e1, 4=b_e2, 5=bq/sqrt(DK)
            _bc = [(load_const(f"bcol{i}a", biases_h[i, 0:128], 128, 1, f32, q=1),
                    load_const(f"bcol{i}b", biases_h[i, 128:256], 128, 1, f32, q=1))
                   for i in range(6)]

            class _BCol:
                def __init__(self, pair):
                    self.pair = pair

                def __getitem__(self, s):
                    lo, hi = s.start or 0, s.stop
                    c, r = divmod(lo, 128)
                    assert hi - lo <= 128 - r
                    return self.pair[c][r : r + (hi - lo)]

            bcol = [_BCol(p) for p in _bc]
            lng = [load_const(f"lng{c}", lng_h[c * 128 : (c + 1) * 128], 128, 1, f32, q=1)
                   for c in range(2)]
            lnb = [load_const(f"lnb{c}", lnb_h[c * 128 : (c + 1) * 128], 128, 1, f32, q=1)
                   for c in range(2)]
            # ---------------- persistent big tensors ----------------
            mask_bf = big.tile([128, KC * T], bf16, name="mask_bf")
            # head h lives at partitions 32*(h%4) of group g=h//4
            kT_bf = [big.tile([128, KV], bf16, name=f"kT{g}") for g in range(2)]
            qT_bf = [big.tile([128, T], bf16, name=f"qT{g}") for g in range(2)]
            v_aug = big.tile([128, KC * H * HD], bf16, name="v_aug")
            xq_sb = [big.tile([128, D], f32, name=f"xq{t}") for t in range(4)]
            xnT_bf = [big.tile([128, T], bf16, name=f"xnT{c}") for c in range(2)]

            va4 = v_aug.rearrange("p (c h e) -> p c h e", c=KC, h=H)
            nc.vector.memset(va4[:, :, :, DK : DK + 1], 1.0)
            va3 = v_aug.rearrange("p (c e) -> p c e", c=KC)  # [128, KC, 264]

            if do_a and not do_p:
                for t_ in kT_bf + qT_bf:
                    nc.vector.memset(t_[:], 0.5)
                nc.vector.memset(v_aug[:], 0.5)
                nc.vector.memset(va4[:, :, :, DK : DK + 1], 1.0)
            if do_a and not do_m:
                nc.vector.memset(mask_bf[:], 1.0)
            if do_epi and not do_l:
                for t_ in xq_sb:
                    nc.vector.memset(t_[:], 0.0)

            for _rep in range(max(1, loop)):
                # ====== phase A+B: input DMAs, LN, q, per-pair prep ======
                with tc.tile_pool(name="pp", bufs=1, space="PSUM") as pp:
                    if do_l:
                        for t in range(4):
                            nc.sync.dma_start(
                                out=xq_sb[t][:], in_=xq_h[t * 128 : (t + 1) * 128]
                            )
                    if do_m:
                        for mc in range(4):
                            cs = slice(mc * 8 * T, (mc + 1) * 8 * T)
                            nc.sync.dma_start(out=mask_bf[:, cs], in_=maskt_h[:, cs])

                    for t in range(4 if do_l else 0):
                        xt = xq_sb[t]
                        mu = prep.tile([128, 1], f32, name="ln_mu", tag="lncol", bufs=8)
                        nc.vector.tensor_reduce(
                            mu[:], xt[:], mybir.AxisListType.X, OP.add
                        )
                        nc.vector.tensor_scalar_mul(mu[:], mu[:], 1.0 / D)
                        xc = prep.tile([128, D], f32, name="ln_xc", tag="lnw", bufs=4)
                        nc.vector.tensor_scalar(xc[:], xt[:], mu[:], None, OP.subtract)
                        sq = prep.tile([128, D], f32, name="ln_sq", tag="lnw", bufs=4)
                        var = prep.tile([128, 1], f32, name="ln_var", tag="lncol", bufs=8)
                        nc.scalar.activation(sq[:], xc[:], AF.Square, accum_out=var[:])
                        # 1/sqrt(var/D + eps) = exp(-0.5 * ln(var/D + eps));
                        # keeps ACT on the single natural_log_exp table set.
                        lnv = prep.tile([128, 1], f32, name="ln_lnv", tag="lncol", bufs=8)
                        nc.scalar.activation(
                            lnv[:], var[:], AF.Ln, bias=eps_col[:], scale=1.0 / D
                        )
                        rstd = prep.tile([128, 1], f32, name="ln_rstd", tag="lncol", bufs=8)
                        nc.scalar.activation(rstd[:], lnv[:], AF.Exp, scale=-0.5)
                        xn0 = prep.tile([128, D], f32, name="ln_xn0", tag="lnw", bufs=4)
                        nc.vector.tensor_scalar_mul(xn0[:], xc[:], rstd[:])
                        for c in range(2):
                            tp = pp.tile([128, 512], f32, name="tp", tag="v", bufs=2)
                            nc.tensor.transpose(
                                tp[:, 0:128], xn0[:, c * 128 : (c + 1) * 128], ident[:]
                            )
                            nc.vector.tensor_scalar(
                                xnT_bf[c][:, t * 128 : (t + 1) * 128],
                                tp[:, 0:128], lng[c][:], lnb[c][:], OP.mult, OP.add,
                            )
                    for g in range(2 if do_l else 0):
                        qp = pp.tile([128, 1024], f32, name="qp", tag="st", bufs=3)
                        gs = slice(g * 128, (g + 1) * 128)
                        nc.tensor.matmul(qp[:, 0:T], w_q[0][:, gs], xnT_bf[0][:],
                                         start=True, stop=False)
                        nc.tensor.matmul(qp[:, 0:T], w_q[1][:, gs], xnT_bf[1][:],
                                         start=False, stop=True)
                        nc.vector.tensor_scalar_add(
                            qT_bf[g][:], qp[:, 0:T], bcol[5][g * 128 : (g + 1) * 128]
                        )

                    def mm2(ps, terms, rows=slice(0, 128)):
                        """Accumulate matmul terms into both 512-halves of a
                        [128,1024] psum tile (one bank per matmul)."""
                        for j in range(2):
                            for idx, (lh, mv) in enumerate(terms):
                                nc.tensor.matmul(
                                    ps[rows, j * T : (j + 1) * T],
                                    lh, mv[:, j * T : (j + 1) * T],
                                    start=(idx == 0), stop=(idx == len(terms) - 1),
                                )

                    for np_ in range(NP if do_p else 0):
                        n0 = 2 * np_
                        nts2 = slice(n0 * T, (n0 + 2) * T)  # 2-neighbor slice
                        md2 = md_all[:, nts2]
                        na2 = na_all[:, nts2]
                        ea2 = ea_all[:, nts2]

                        # ts branch: ts1 = relu(W_ts1^T md + b)
                        ts1 = [prep.tile([128, 1024], bf16, name=f"ts1_{c}",
                                         tag=f"ts1{c}", bufs=2) for c in range(2)]
                        ps = pp.tile([128, 1024], f32, name="ps_ts1a", tag="st", bufs=3)
                        mm2(ps, [(w_ts1[:, 0:128], md2)])
                        nc.scalar.activation(ts1[0][:], ps[:], AF.Relu, bias=bcol[0][0:128])
                        ps = pp.tile([128, 1024], f32, name="ps_ts1b", tag="st", bufs=3)
                        mm2(ps, [(w_ts1[:, 128:TSE], md2)], rows=slice(0, TSE - 128))
                        nc.scalar.activation(
                            ts1[1][0 : TSE - 128, :], ps[0 : TSE - 128, :],
                            AF.Relu, bias=bcol[0][128:TSE])
                        # aux branch: a1 = relu(W_a1^T na + b)
                        a1 = prep.tile([128, 1024], bf16, name="a1sb", tag="a1", bufs=2)
                        ps = pp.tile([128, 1024], f32, name="ps_a1", tag="st", bufs=3)
                        mm2(ps, [(w_a1[:], na2)], rows=slice(0, AUXE))
                        nc.scalar.activation(
                            a1[0:AUXE, :], ps[0:AUXE, :], AF.Relu, bias=bcol[2][0:AUXE])

                        # nbr = [ts2 (192) ; a2 (64)] as 2 chunks of 128
                        nbr = [prep.tile([128, 1024], bf16, name=f"nbr_{c}",
                                         tag=f"nbr{c}", bufs=2) for c in range(2)]
                        ps = pp.tile([128, 1024], f32, name="ps_nbr0", tag="st", bufs=3)
                        mm2(ps, [(w_ts2[0][:, 0:128], ts1[0]),
                                 (w_ts2[1][0:64, 0:128], ts1[1][0:64, :])])
                        nc.vector.tensor_scalar_add(nbr[0][:], ps[:], bcol[1][0:128])
                        ps = pp.tile([128, 1024], f32, name="ps_nbr1", tag="st", bufs=3)
                        mm2(ps, [(w_ts2[0][:, 128:TSE], ts1[0]),
                                 (w_ts2[1][0:64, 128:TSE], ts1[1][0:64, :])],
                            rows=slice(0, 64))
                        nc.vector.tensor_scalar_add(
                            nbr[1][0:64, :], ps[0:64, :], bcol[1][128:TSE])
                        ps = pp.tile([128, 1024], f32, name="ps_a2", tag="st", bufs=3)
                        mm2(ps, [(w_a2[:], a1[0:AUXE, :])], rows=slice(0, AUXE))
                        nc.vector.tensor_scalar_add(
                            nbr[1][64:128, :], ps[0:AUXE, :], bcol[2][64:128])

                        # edge branch
                        e1 = [prep.tile([128, 1024], bf16, name=f"e1_{c}",
                                        tag=f"e1{c}", bufs=2) for c in range(2)]
                        for c in range(2):
                            ps = pp.tile([128, 1024], f32, name="ps_e1", tag="st", bufs=3)
                            mm2(ps, [(w_e1[:, c * 128 : (c + 1) * 128], ea2)])
                            nc.scalar.activation(
                                e1[c][:], ps[:], AF.Relu,
                                bias=bcol[3][c * 128 : (c + 1) * 128])
                        e2 = [prep.tile([128, 1024], bf16, name=f"e2_{c}",
                                        tag=f"e2{c}", bufs=2) for c in range(2)]
                        for c in range(2):
                            cs = slice(c * 128, (c + 1) * 128)
                            ps = pp.tile([128, 1024], f32, name="ps_e2", tag="st", bufs=3)
                            mm2(ps, [(w_e2[0][:, cs], e1[0]), (w_e2[1][:, cs], e1[1])])
                            nc.vector.tensor_scalar_add(e2[c][:], ps[:], bcol[4][cs])

                        # keysT = nbr * e2 (pe+bias via pewk const)
                        keys = [prep.tile([128, 1024], bf16, name=f"keys_{c}",
                                          tag=f"keys{c}", bufs=1) for c in range(2)]
                        nc.gpsimd.tensor_tensor(keys[0][:], nbr[0][:], e2[0][:], OP.mult)
                        nc.gpsimd.tensor_tensor(keys[1][:], nbr[1][:], e2[1][:], OP.mult)

                        # kT = Wk^T keys + pewk^T -> [128, KV] per group
                        for g in range(2):
                            gs = slice(g * 128, (g + 1) * 128)
                            ps = pp.tile([128, 1024], f32, name="ps_kt", tag="st", bufs=3)
                            mm2(ps, [(w_k[0][:, gs], keys[0]), (w_k[1][:, gs], keys[1])])
                            nc.vector.tensor_tensor(
                                kT_bf[g][:, nts2], ps[:], pewk[g][:], OP.add)

                        # v rows: v = nbr^T Wv (+ pewv via 4-dim AP add)
                        for i in range(2):
                            for j2 in range(2):
                                ps = pp.tile([128, 512], f32, name="ps_v", tag="v", bufs=2)
                                for jj in range(2):
                                    j = 2 * j2 + jj
                                    ts_ = slice(i * 512 + j * 128, i * 512 + (j + 1) * 128)
                                    nc.tensor.matmul(
                                        ps[:, jj * D : (jj + 1) * D], nbr[0][:, ts_],
                                        w_v[0][:], start=True, stop=False)
                                    nc.tensor.matmul(
                                        ps[:, jj * D : (jj + 1) * D], nbr[1][:, ts_],
                                        w_v[1][:], start=False, stop=True)
                                kc0 = (n0 + i) * 4 + 2 * j2
                                nc.vector.tensor_tensor(
                                    va4[:, kc0 : kc0 + 2, :, 0:DK],
                                    ps[:].rearrange("p (j h e) -> p j h e", j=2, h=H),
                                    pewv4[:, 2 * j2 * D : (2 * j2 + 2) * D].rearrange(
                                        "p (j h e) -> p j h e", j=2, h=H),
                                    OP.add,
                                )

                # ====== phase C: pipelined packed attention ======
                if do_a:
                    with tc.tile_pool(name="pcx", bufs=1, space="PSUM") as pcx:
                        # bank j: head 2j at rows 0:33, head 2j+1 at rows 64:97
                        cxb = [pcx.tile([128, T], f32, name=f"cxb{j}")
                               for j in range(4)]
                        with tc.tile_pool(name="psc", bufs=1, space="PSUM") as psc:
                            for kc in range(KC):
                                p0 = ppool.tile([128, H * T], bf16, name="p0",
                                                tag="p0", bufs=2)
                                pm = ppool.tile([128, H * T], bf16, name="pm",
                                                tag="pm", bufs=2)
                                for G in range(4):
                                    g, pr = divmod(G, 2)
                                    sp = psc.tile([128, 2 * T], f32, name="sp",
                                                  tag="sp", bufs=2)
                                    for i in range(2):
                                        po = 64 * pr + 32 * i
                                        nc.tensor.matmul(
                                            sp[:, i * T : (i + 1) * T],
                                            kT_bf[g][po : po + 32,
                                                     kc * 128 : (kc + 1) * 128],
                                            qT_bf[g][po : po + 32, :],
                                            start=True, stop=True,
                                            tile_position=(po, 0),
                                        )
                                    nc.scalar.activation(
                                        p0[:, G * 2 * T : (G + 1) * 2 * T], sp[:], AF.Exp
                                    )
                                mbc = mask_bf[:, kc * T : (kc + 1) * T].rearrange(
                                    "p (o t) -> p o t", o=1
                                ).to_broadcast((128, 4, T))
                                for half in range(2):
                                    hs = slice(half * 4 * T, (half + 1) * 4 * T)
                                    nc.vector.tensor_tensor(
                                        pm[:, hs].rearrange("p (o t) -> p o t", o=4),
                                        p0[:, hs].rearrange("p (o t) -> p o t", o=4),
                                        mbc, OP.mult,
                                    )
                                for j in range(4):
                                    for i in range(2):
                                        h = 2 * j + i
                                        nc.tensor.matmul(
                                            cxb[j][64 * i : 64 * i + HD, :],
                                            va3[:, kc, h * HD : (h + 1) * HD],
                                            pm[:, h * T : (h + 1) * T],
                                            start=(kc == 0), stop=(kc == KC - 1),
                                        )

                        # ====== phase D: z AllReduce + epilogue ======
                        if do_epi:
                            with tc.tile_pool(name="px", bufs=2, space="PSUM") as px:
                                cc_in = dram.tile([H * T], f32, name="cc_in")
                                cc_out = dram.tile([H * T], f32, name="cc_out")
                                # z row for head h at partition 32*(h%4), col-half h//4
                                z_tile = big.tile([128, 2 * T], f32, name="z_tile")
                                for j in range(4):
                                    for i in range(2):
                                        h = 2 * j + i
                                        nc.vector.tensor_copy(
                                            z_tile[32 * (h % 4) : 32 * (h % 4) + 1,
                                                   (h // 4) * T : (h // 4 + 1) * T],
                                            cxb[j][64 * i + DK : 64 * i + DK + 1, :],
                                        )
                                zv = z_tile.rearrange("(a b) (c t) -> a b c t", b=32, c=2)
                                nc.sync.dma_start(
                                    out=cc_in.rearrange("(c a t) -> a c t", a=4, c=2),
                                    in_=zv[:, 0, :, :],
                                )
                                if no_collective:
                                    nc.sync.dma_start(out=cc_out[:], in_=cc_in[:])
                                else:
                                    nc.gpsimd.collective_compute(
                                        "AllReduce", OP.add, replica_groups=RG,
                                        ins=[cc_in[:]], outs=[cc_out[:]],
                                    )
                                zg8 = big.tile([8, T], f32, name="zg8")
                                nc.sync.dma_start(
                                    out=zg8[:],
                                    in_=cc_out.rearrange("(h t) -> h t", h=H),
                                )
                                rzf8 = big.tile([8, T], bf16, name="rzf8")
                                ctxn = [big.tile([128, T], bf16, name=f"ctxn{c}")
                                        for c in range(2)]
                                with nc.allow_low_precision(reason="1/z bf16; tol 2e-2"):
                                    nc.vector.reciprocal(rzf8[:], zg8[:])
                                    for c4 in range(2):
                                        # 1/z broadcast rows for heads 4*c4..4*c4+3,
                                        # col-packed 4x into one psum bank
                                        bc = px.tile([128, T], f32, name="bc", tag="px")
                                        for r4 in range(4):
                                            h = 4 * c4 + r4
                                            nc.tensor.matmul(
                                                bc[r4 * DK : (r4 + 1) * DK, :],
                                                ind8[:, h * DK : (h + 1) * DK],
                                                rzf8[:], start=True, stop=True,
                                                tile_position=(0, r4 * DK),
                                            )
                                        bcs = prep.tile([128, T], bf16, name="bcs",
                                                        tag="bcs", bufs=2)
                                        nc.vector.tensor_copy(bcs[:], bc[:])
                                        for r4 in range(4):
                                            h = 4 * c4 + r4
                                            nc.vector.tensor_tensor(
                                                ctxn[c4][r4 * DK : (r4 + 1) * DK, :],
                                                cxb[h // 2][64 * (h % 2) : 64 * (h % 2) + DK, :],
                                                bcs[r4 * DK : (r4 + 1) * DK, :], OP.mult)
                                for t in range(4):
                                    ts_ = slice(t * 128, (t + 1) * 128)
                                    op_ = px.tile([128, D], f32, name="op", tag="po")
                                    nc.tensor.matmul(op_[:], ctxn[0][:, ts_], w_o[0][:],
                                                     start=True, stop=False)
                                    nc.tensor.matmul(op_[:], ctxn[1][:, ts_], w_o[1][:],
                                                     start=False, stop=False)
                                    nc.tensor.matmul(op_[:], ones_row[0:1, ts_], w_o_b[:],
                                                     start=False, stop=True)
                                    ot = prep.tile([128, D], f32, name="out_sb",
                                                   tag="lnw", bufs=4)
                                    nc.vector.tensor_add(ot[:], op_[:], xq_sb[t][:])
                                    nc.sync.dma_start(out=out_h[ts_, :], in_=ot[:])

    nc.finalize()
    return nc


def _host_inputs(inputs):
    """Build the 8 per-core input maps from full inputs."""
    import ml_dtypes

    bf16 = ml_dtypes.bfloat16
    pe = _pe_table()
    sc = np.float32(1.0 / math.sqrt(DK))

    w = {k: np.asarray(v, dtype=np.float32) if np.asarray(v).dtype != np.int32
         else np.asarray(v) for k, v in inputs.items()}

    def pad_col(v):
        out = np.zeros((D, 1), np.float32)
        out[: v.shape[0], 0] = v
        return out

    biases = np.stack([
        pad_col(w["b_ts1"]),
        pad_col(w["b_ts2"]),
        pad_col(np.concatenate([w["b_a1"], w["b_a2"]])),
        pad_col(w["b_e1"]),
        pad_col(w["b_e2"]),
        pad_col(w["bq"] * sc),
    ])

    w1pack = np.zeros((64 + CE, D), np.float32)
    w1pack[0:CTS, 0:TSE] = w["W_ts1"]
    w1pack[32 : 32 + CN, 0:AUXE] = w["W_a1"]
    w1pack[64 : 64 + CE, 0:D] = w["W_e1"]

    pewv = (pe @ w["Wv"] + w["bv"]).astype(np.float32)  # [T, D]

    shared = {
        "w1pack": w1pack.astype(bf16),
        "wts2": w["W_ts2"].astype(bf16),
        "wa2": w["W_a2"].astype(bf16),
        "we2": w["W_e2"].astype(bf16),
        "wq": (w["Wq"] * sc).astype(bf16),
        "wk": w["Wk"].astype(bf16),
        "wv": w["Wv"].astype(bf16),
        "biases": biases,
        "pewk": np.ascontiguousarray(
            np.tile((pe @ w["Wk"] + w["bk"]).T, (1, 2))
        ).astype(bf16),
        "pewv4": np.ascontiguousarray(
            pewv.reshape(4, 128, D).transpose(1, 0, 2).reshape(128, 4 * D)
        ).astype(bf16),
        "lng": w["ln_g"].reshape(D, 1).astype(np.float32),
        "lnb": w["ln_b"].reshape(D, 1).astype(np.float32),
        "ind8": np.kron(np.eye(H, dtype=np.float32),
                        np.ones((1, DK), np.float32)).astype(bf16),
    }
    wo_even = np.concatenate([w["Wo"], w["bo"][None, :]], axis=0).astype(bf16)
    wo_odd = np.concatenate([w["Wo"], np.zeros((1, D), np.float32)], axis=0).astype(bf16)

    in_maps = []
    for c in range(NCORES):
        b, half = divmod(c, 2)
        n0 = half * NBH
        m = dict(shared)
        # half-scaled: LN is scale-invariant (with eps/4 on device) and both
        # pair cores add it as residual, so the host pair-sum restores 1.0x.
        m["xq"] = (0.5 * (w["x"][b] + pe)).astype(np.float32)
        aux = np.zeros((64 + CE, NBH * T), np.float32)
        aux[0:CTS] = w["masked_data"][b, n0 : n0 + NBH].transpose(1, 0, 2).reshape(CTS, -1)
        aux[32 : 32 + CN] = w["node_aux"][b, n0 : n0 + NBH].transpose(1, 0, 2).reshape(CN, -1)
        aux[64 : 64 + CE] = w["edge_aux"][b, n0 : n0 + NBH].transpose(1, 0, 2).reshape(CE, -1)
        m["aux"] = aux.astype(bf16)
        # mask in on-chip layout: maskt[p, c*T + t] = mask[c*128+p, t]
        msk = w["attention_mask"][b, :, half * KV : (half + 1) * KV].T  # [KV, T]
        m["maskt"] = np.ascontiguousarray(
            msk.reshape(KC, 128, T).transpose(1, 0, 2).reshape(128, KC * T)
        ).astype(bf16)
        m["wo"] = wo_even if half == 0 else wo_odd
        in_maps.append(m)
    return in_maps


def _get_nc():
    if "nc" not in _CACHE:
        _CACHE["nc"] = build_nc()
    return _CACHE["nc"]


def kernel(**inputs) -> np.ndarray:
    from concourse.bass_utils import run_bass_kernel_spmd

    nc = _get_nc()
    in_maps = _host_inputs(inputs)
    res = run_bass_kernel_spmd(nc, in_maps, list(range(NCORES)))
    out = np.stack(
        [res.results[2 * b]["out"] + res.results[2 * b + 1]["out"] for b in range(B)],
        axis=0,
    )
    return out.astype(np.float32)
